# revision 5
# baseline (speedup 1.0000x reference)
"""CrossModalAttention Trainium2 Bass kernel (8-core data parallel).

Math notes (matches the fp32 jax reference exactly up to float rounding):
  - nn.MultiheadAttention with q_len = kv_len = 1: softmax over a single key
    is identically 1.0, so attn(q, kv) = (kv @ Wv.T + bv) @ Wo.T + bo
    = kv @ (Wo @ Wv).T + (Wo @ bv + bo).  Wq / Wk drop out entirely.
  - Per branch (img / text):
        z1 = x + x_other @ Weff.T + beff
        y1 = LN1(z1)                       (gamma=1, beta=0 in this problem)
        z2 = y1 + gelu(y1 @ W1.T + b1) @ W2.T + b2
        y2 = LN2(z2)                       -> output
Device layout: activations row-major (rows on SBUF partitions), LN via
bn_stats/bn_aggr + fused tensor_scalar.  Matmuls are activation-stationary
(lhsT = transposed activations): attn uses host-pre-transposed x, FFN2 uses
the feature-major gelu output directly, only y1 is transposed on device
(bf16 DMA transpose).  Matmul inputs bf16, everything else fp32.
"""

import os
import sys

import numpy as np

for _p in ("/opt/trn_rl_repo", "/root/.axon_site/_ro/trn_rl_repo",
           "/root/.axon_site", "/root/.axon_site/_ro/pypackages"):
    if os.path.isdir(_p) and _p not in sys.path:
        sys.path.append(_p)

import ml_dtypes

BF16 = ml_dtypes.bfloat16

B, E, FF, H = 65536, 512, 1024, 8
N_CORES = 8
RPC = B // N_CORES          # rows per core
ST = 512                    # supertile rows
RC = ST // 128              # rowchunks per supertile
EC = E // 128               # feature chunks (4)
FC = FF // 128              # ffn feature chunks (8)
EPS = 1e-5

_BUILD_CACHE = {}


def _build_program(rows, reps=1, real_gelu=True):
    """Build + compile the Bass program for one core processing `rows` rows."""
    import concourse.bass as bass  # noqa: F401
    import concourse.mybir as mybir
    from concourse import bacc, tile

    dt = mybir.dt
    f32, bf = dt.float32, dt.bfloat16
    nst = rows // ST
    assert rows % ST == 0

    nc = bacc.Bacc("TRN2", target_bir_lowering=False, debug=False,
                   enable_asserts=False, num_devices=1)

    def din(name, shape, d=f32):
        return nc.dram_tensor(name, shape, d, kind="ExternalInput").ap()

    mods = ("text", "img")
    xT = {m: din(f"xT_{m}", (E, rows), bf) for m in mods}
    xp = {m: din(f"xp_{m}", (rows, E), f32) for m in mods}
    wefft = {m: din(f"wefft_{m}", (E, E), bf) for m in mods}
    w1t = {m: din(f"w1t_{m}", (E, FF), bf) for m in mods}
    w2t = {m: din(f"w2t_{m}", (FF, E), bf) for m in mods}
    b1 = {m: din(f"b1_{m}", (128, FC), f32) for m in mods}
    b2 = {m: din(f"b2_{m}", (1, E), bf) for m in mods}
    out = {m: nc.dram_tensor(f"out_{m}", (rows, E), f32, kind="ExternalOutput").ap()
           for m in mods}
    other = {"img": "text", "text": "img"}

    with tile.TileContext(nc) as tc:
        with (
            tc.tile_pool(name="wpool", bufs=1) as wpool,
            tc.tile_pool(name="xTp", bufs=4) as xTp,
            tc.tile_pool(name="xpp", bufs=4) as xpp,
            tc.tile_pool(name="y1Tp", bufs=2) as y1Tp,
            tc.tile_pool(name="hp", bufs=2) as hp,
            tc.tile_pool(name="zp", bufs=4) as zp,
            tc.tile_pool(name="yp", bufs=10) as yp,
            tc.tile_pool(name="ybp", bufs=4) as ybp,
            tc.tile_pool(name="y2p", bufs=4) as y2p,
            tc.tile_pool(name="stp", bufs=16) as stp,
            tc.tile_pool(name="pa", bufs=2, space="PSUM") as pa_pool,
            tc.tile_pool(name="pu", bufs=2, space="PSUM") as pu_pool,
            tc.tile_pool(name="pf", bufs=2, space="PSUM") as pf_pool,
        ):
            # ---- resident weights ----
            wefft_sb, w1t_sb, w2t_sb, b1_sb, b2_sb = {}, {}, {}, {}, {}
            for m in mods:
                wefft_sb[m] = wpool.tile([128, EC, E], bf, tag=f"wefft_{m}", name=f"wefft_{m}_sb")
                nc.sync.dma_start(
                    wefft_sb[m][:], wefft[m].rearrange("(kc p) n -> p kc n", p=128))
                w1t_sb[m] = wpool.tile([128, EC, FF], bf, tag=f"w1t_{m}", name=f"w1t_{m}_sb")
                nc.sync.dma_start(
                    w1t_sb[m][:], w1t[m].rearrange("(kc p) n -> p kc n", p=128))
                w2t_sb[m] = wpool.tile([128, FC, E], bf, tag=f"w2t_{m}", name=f"w2t_{m}_sb")
                nc.sync.dma_start(
                    w2t_sb[m][:], w2t[m].rearrange("(kc p) n -> p kc n", p=128))
                b1_sb[m] = wpool.tile([128, FC], f32, tag=f"b1_{m}", name=f"b1_{m}_sb")
                nc.sync.dma_start(b1_sb[m][:], b1[m])
                b2_sb[m] = wpool.tile([1, E], bf, tag=f"b2_{m}", name=f"b2_{m}_sb")
                nc.sync.dma_start(b2_sb[m][:], b2[m])
            ones_sb = wpool.tile([1, 128], bf, tag="ones")
            nc.vector.memset(ones_sb[:], 1.0)
            eps_sb = wpool.tile([128, 1], f32, tag="eps")
            nc.vector.memset(eps_sb[:], EPS)

            def body(_iv=None):
                for st in range(nst):
                    r0 = st * ST
                    xT_sb, xp_sb = {}, {}
                    for m in mods:
                        xT_sb[m] = xTp.tile([128, EC, ST], bf, tag="xT", name=f"xT_{m}_sb")
                        nc.sync.dma_start(
                            xT_sb[m][:],
                            xT[m][:, r0:r0 + ST].rearrange("(kc p) n -> p kc n", p=128))
                        xp_sb[m] = xpp.tile([128, RC, E], f32, tag="xp", name=f"xp_{m}_sb")
                        nc.sync.dma_start(
                            xp_sb[m][:],
                            xp[m][r0:r0 + ST, :].rearrange("(r p) f -> p r f", p=128))

                    for m in mods:          # branch producing out[m]
                        o = other[m]
                        y1 = []
                        y1T_sb = y1Tp.tile([128, EC, ST], bf, tag="y1T")
                        # --- attn + LN1 per rowchunk ---
                        for r in range(RC):
                            p_a = pa_pool.tile([128, E], f32, tag="pa")
                            for kc in range(EC):
                                nc.tensor.matmul(
                                    p_a[:],
                                    xT_sb[o][:, kc, r * 128:(r + 1) * 128],
                                    wefft_sb[m][:, kc, :],
                                    start=(kc == 0), stop=(kc == EC - 1))
                            z1 = zp.tile([128, E], f32, tag="z1")
                            nc.vector.tensor_add(z1[:], p_a[:], xp_sb[m][:, r, :])
                            st6 = stp.tile([128, 6], f32, tag="st6")
                            nc.vector.bn_stats(st6[:], z1[:])
                            mv = stp.tile([128, 2], f32, tag="mv")
                            nc.vector.bn_aggr(mv[:], st6[:])
                            rs = stp.tile([128, 1], f32, tag="rs")
                            nc.scalar.activation(
                                rs[:], mv[:, 1:2],
                                mybir.ActivationFunctionType.Sqrt, bias=eps_sb[:])
                            nc.vector.reciprocal(rs[:], rs[:])
                            y1r = yp.tile([128, E], f32, tag="y1")
                            nc.vector.tensor_scalar(
                                y1r[:], z1[:], mv[:, 0:1], rs[:],
                                mybir.AluOpType.subtract, mybir.AluOpType.mult)
                            y1.append(y1r)
                            y1b = ybp.tile([128, E], bf, tag="y1b")
                            nc.scalar.copy(y1b[:], y1r[:])
                            for kc in range(EC):
                                nc.sync.dma_start_transpose(
                                    y1T_sb[:, kc, r * 128:(r + 1) * 128],
                                    y1b[:, kc * 128:(kc + 1) * 128])
                        # --- FFN1 + gelu (feature-major out) ---
                        h_sb = hp.tile([128, FC, ST], bf, tag="h")
                        for mc in range(FC):
                            p_u = pu_pool.tile([128, ST], f32, tag="pu")
                            for kc in range(EC):
                                nc.tensor.matmul(
                                    p_u[:],
                                    w1t_sb[m][:, kc, mc * 128:(mc + 1) * 128],
                                    y1T_sb[:, kc, :],
                                    start=(kc == 0), stop=(kc == EC - 1))
                            nc.scalar.activation(
                                h_sb[:, mc, :], p_u[:],
                                mybir.ActivationFunctionType.Gelu if real_gelu
                                else mybir.ActivationFunctionType.Identity,
                                bias=b1_sb[m][:, mc:mc + 1])
                        # --- FFN2 + LN2 per rowchunk ---
                        for r in range(RC):
                            p_f = pf_pool.tile([128, E], f32, tag="pf")
                            for kc in range(FC):
                                nc.tensor.matmul(
                                    p_f[:],
                                    h_sb[:, kc, r * 128:(r + 1) * 128],
                                    w2t_sb[m][:, kc, :],
                                    start=(kc == 0), stop=False)
                            nc.tensor.matmul(
                                p_f[:], ones_sb[:], b2_sb[m][:],
                                start=False, stop=True)
                            z2 = zp.tile([128, E], f32, tag="z2")
                            nc.vector.tensor_add(z2[:], p_f[:], y1[r][:])
                            st6 = stp.tile([128, 6], f32, tag="st6")
                            nc.vector.bn_stats(st6[:], z2[:])
                            mv = stp.tile([128, 2], f32, tag="mv")
                            nc.vector.bn_aggr(mv[:], st6[:])
                            rs = stp.tile([128, 1], f32, tag="rs")
                            nc.scalar.activation(
                                rs[:], mv[:, 1:2],
                                mybir.ActivationFunctionType.Sqrt, bias=eps_sb[:])
                            nc.vector.reciprocal(rs[:], rs[:])
                            y2 = y2p.tile([128, E], f32, tag="y2")
                            nc.vector.tensor_scalar(
                                y2[:], z2[:], mv[:, 0:1], rs[:],
                                mybir.AluOpType.subtract, mybir.AluOpType.mult)
                            nc.sync.dma_start(
                                out[m][r0 + r * 128:r0 + (r + 1) * 128, :], y2[:])

            if reps == 1:
                body()
            else:
                with tc.For_i(0, reps, 1) as iv:
                    body(iv)

    nc.compile()
    return nc


def _host_prepare(inputs):
    """Fold weights, shard the batch, build per-core input maps."""
    f64 = np.float64

    def fold(p):
        Wv, bv = inputs[f"{p}_Wv"], inputs[f"{p}_bv"]
        Wo, bo = inputs[f"{p}_Wo"], inputs[f"{p}_bo"]
        weff = (Wo.astype(f64) @ Wv.astype(f64)).astype(np.float32)
        beff = (Wo.astype(f64) @ bv.astype(f64) + bo.astype(f64)).astype(np.float32)
        return weff, beff

    weff_i2t, beff_i2t = fold("i2t")   # img branch: img queries text context
    weff_t2i, beff_t2i = fold("t2i")   # text branch

    const = {}
    br_w = {"img": (weff_i2t, beff_i2t, "ffn_img"),
            "text": (weff_t2i, beff_t2i, "ffn_text")}
    for m, (weff, beff, ffn) in br_w.items():
        const[f"wefft_{m}"] = np.ascontiguousarray(weff.T).astype(BF16)
        const[f"w1t_{m}"] = np.ascontiguousarray(inputs[f"{ffn}_W1"].T).astype(BF16)
        const[f"w2t_{m}"] = np.ascontiguousarray(inputs[f"{ffn}_W2"].T).astype(BF16)
        const[f"b1_{m}"] = np.ascontiguousarray(
            inputs[f"{ffn}_b1"].reshape(FC, 128).T).astype(np.float32)
        const[f"b2_{m}"] = inputs[f"{ffn}_b2"].reshape(1, E).astype(BF16)

    text = np.asarray(inputs["text_feat"], dtype=np.float32)
    img = np.asarray(inputs["img_feat"], dtype=np.float32)
    in_maps = []
    for c in range(N_CORES):
        s = slice(c * RPC, (c + 1) * RPC)
        ts, is_ = text[s], img[s]
        m = dict(const)
        m["xT_text"] = np.ascontiguousarray(ts.T).astype(BF16)
        m["xT_img"] = np.ascontiguousarray(is_.T).astype(BF16)
        m["xp_img"] = is_ + beff_i2t[None, :]
        m["xp_text"] = ts + beff_t2i[None, :]
        in_maps.append(m)
    return in_maps


def _ln_affine_trivial(inputs):
    for n in ("ln1i", "ln1t", "ln2i", "ln2t"):
        if not np.allclose(inputs[f"{n}_g"], 1.0, atol=0.0):
            return False
        if not np.allclose(inputs[f"{n}_b"], 0.0, atol=0.0):
            return False
    return True


def _reference_numpy(inputs):
    """Exact numpy fallback (only used if LN affine params are nontrivial)."""
    from scipy.special import erf  # pragma: no cover

    def ln(x, g, b, eps=EPS):
        m = x.mean(-1, keepdims=True)
        xc = x - m
        v = (xc * xc).mean(-1, keepdims=True)
        return xc / np.sqrt(v + eps) * g + b

    def mha1(q, kv, p):
        vh = kv @ inputs[f"{p}_Wv"].T + inputs[f"{p}_bv"]
        return vh @ inputs[f"{p}_Wo"].T + inputs[f"{p}_bo"]

    def ffn(x, p):
        u = x @ inputs[f"{p}_W1"].T + inputs[f"{p}_b1"]
        h = 0.5 * u * (1.0 + erf(u / np.sqrt(2.0)))
        return h @ inputs[f"{p}_W2"].T + inputs[f"{p}_b2"]

    text, img = inputs["text_feat"], inputs["img_feat"]
    img_out = ln(img + mha1(img, text, "i2t"), inputs["ln1i_g"], inputs["ln1i_b"])
    text_out = ln(text + mha1(text, img, "t2i"), inputs["ln1t_g"], inputs["ln1t_b"])
    img_out = ln(img_out + ffn(img_out, "ffn_img"), inputs["ln2i_g"], inputs["ln2i_b"])
    text_out = ln(text_out + ffn(text_out, "ffn_text"),
                  inputs["ln2t_g"], inputs["ln2t_b"])
    return text_out.astype(np.float32), img_out.astype(np.float32)


def kernel(**inputs):
    inputs = {k: np.asarray(v) for k, v in inputs.items()}
    if not _ln_affine_trivial(inputs):
        return _reference_numpy(inputs)

    from concourse.bass_utils import run_bass_kernel_spmd

    key = ("main", RPC, 1)
    if key not in _BUILD_CACHE:
        _BUILD_CACHE[key] = _build_program(RPC, reps=1)
    nc = _BUILD_CACHE[key]

    in_maps = _host_prepare(inputs)
    res = run_bass_kernel_spmd(nc, in_maps, core_ids=list(range(N_CORES)))
    text_out = np.concatenate([res.results[c]["out_text"] for c in range(N_CORES)], 0)
    img_out = np.concatenate([res.results[c]["out_img"] for c in range(N_CORES)], 0)
    return text_out.astype(np.float32), img_out.astype(np.float32)


# revision 8
# speedup vs baseline: 2.3815x; 2.3815x over previous
"""CrossModalAttention Trainium2 Bass kernel (8-core data parallel).

Math notes (matches the fp32 jax reference exactly up to float rounding):
  - nn.MultiheadAttention with q_len = kv_len = 1: softmax over a single key
    is identically 1.0, so attn(q, kv) = (kv @ Wv.T + bv) @ Wo.T + bo
    = kv @ (Wo @ Wv).T + (Wo @ bv + bo).  Wq / Wk drop out entirely.
  - Per branch (img / text):
        z1 = x + x_other @ Weff.T + beff
        y1 = LN1(z1)                       (gamma=1, beta=0 in this problem)
        z2 = y1 + gelu(y1 @ W1.T + b1) @ W2.T + b2
        y2 = LN2(z2)                       -> output
Device layout: activations row-major (rows on SBUF partitions), LN via
bn_stats/bn_aggr + fused tensor_scalar.  Matmuls are activation-stationary
(lhsT = transposed activations): attn uses host-pre-transposed x, FFN2 uses
the feature-major gelu output directly, only y1 is transposed on device
(bf16 DMA transpose).  Matmul inputs bf16, everything else fp32.
"""

import os
import sys

import numpy as np

for _p in ("/opt/trn_rl_repo", "/root/.axon_site/_ro/trn_rl_repo",
           "/root/.axon_site", "/root/.axon_site/_ro/pypackages"):
    if os.path.isdir(_p) and _p not in sys.path:
        sys.path.append(_p)

import ml_dtypes

BF16 = ml_dtypes.bfloat16

B, E, FF, H = 65536, 512, 1024, 8
N_CORES = 8
RPC = B // N_CORES          # rows per core
ST = 512                    # supertile rows
RC = ST // 128              # rowchunks per supertile
EC = E // 128               # feature chunks (4)
FC = FF // 128              # ffn feature chunks (8)
EPS = 1e-5

_BUILD_CACHE = {}


def _build_program(rows, reps=1, real_gelu=True):
    """Build + compile the Bass program for one core processing `rows` rows."""
    import concourse.bass as bass  # noqa: F401
    import concourse.mybir as mybir
    from concourse import bacc, tile

    dt = mybir.dt
    f32, bf = dt.float32, dt.bfloat16
    nst = rows // ST
    assert rows % ST == 0

    nc = bacc.Bacc("TRN2", target_bir_lowering=False, debug=False,
                   enable_asserts=False, num_devices=1)

    def din(name, shape, d=f32):
        return nc.dram_tensor(name, shape, d, kind="ExternalInput").ap()

    mods = ("text", "img")
    xT = {m: din(f"xT_{m}", (E, rows), bf) for m in mods}
    xp = {m: din(f"xp_{m}", (rows, E), f32) for m in mods}
    wefft = {m: din(f"wefft_{m}", (E, E), bf) for m in mods}
    w1t = {m: din(f"w1t_{m}", (E, FF), bf) for m in mods}
    w2t = {m: din(f"w2t_{m}", (FF, E), bf) for m in mods}
    b1 = {m: din(f"b1_{m}", (128, FC), f32) for m in mods}
    b2 = {m: din(f"b2_{m}", (1, E), bf) for m in mods}
    out = {m: nc.dram_tensor(f"out_{m}", (rows, E), f32, kind="ExternalOutput").ap()
           for m in mods}
    other = {"img": "text", "text": "img"}

    with tile.TileContext(nc) as tc:
        with (
            tc.tile_pool(name="wpool", bufs=1) as wpool,
            tc.tile_pool(name="xTp", bufs=4) as xTp,
            tc.tile_pool(name="xpp", bufs=4) as xpp,
            tc.tile_pool(name="y1Tp", bufs=2) as y1Tp,
            tc.tile_pool(name="hp", bufs=2) as hp,
            tc.tile_pool(name="zp", bufs=6) as zp,
            tc.tile_pool(name="scrp", bufs=2) as scrp,
            tc.tile_pool(name="yp", bufs=10) as yp,
            tc.tile_pool(name="ybp", bufs=4) as ybp,
            tc.tile_pool(name="y2p", bufs=4) as y2p,
            tc.tile_pool(name="stp", bufs=16) as stp,
            tc.tile_pool(name="pa", bufs=2, space="PSUM") as pa_pool,
            tc.tile_pool(name="pu", bufs=2, space="PSUM") as pu_pool,
            tc.tile_pool(name="pf", bufs=2, space="PSUM") as pf_pool,
        ):
            # ---- resident weights ----
            wefft_sb, w1t_sb, w2t_sb, b1_sb, b2_sb = {}, {}, {}, {}, {}
            for m in mods:
                wefft_sb[m] = wpool.tile([128, EC, E], bf, tag=f"wefft_{m}", name=f"wefft_{m}_sb")
                nc.sync.dma_start(
                    wefft_sb[m][:], wefft[m].rearrange("(kc p) n -> p kc n", p=128))
                w1t_sb[m] = wpool.tile([128, EC, FF], bf, tag=f"w1t_{m}", name=f"w1t_{m}_sb")
                nc.sync.dma_start(
                    w1t_sb[m][:], w1t[m].rearrange("(kc p) n -> p kc n", p=128))
                w2t_sb[m] = wpool.tile([128, FC, E], bf, tag=f"w2t_{m}", name=f"w2t_{m}_sb")
                nc.sync.dma_start(
                    w2t_sb[m][:], w2t[m].rearrange("(kc p) n -> p kc n", p=128))
                b1_sb[m] = wpool.tile([128, FC], f32, tag=f"b1_{m}", name=f"b1_{m}_sb")
                nc.sync.dma_start(b1_sb[m][:], b1[m])
                b2_sb[m] = wpool.tile([1, E], bf, tag=f"b2_{m}", name=f"b2_{m}_sb")
                nc.sync.dma_start(b2_sb[m][:], b2[m])
            ones_sb = wpool.tile([1, 128], bf, tag="ones")
            nc.vector.memset(ones_sb[:], 1.0)

            ALU = mybir.AluOpType
            INV_E = 1.0 / E
            MAGIC = 0x5F3759DF + 1   # two's-complement: magic - x = ~x + (magic+1)

            def ln_scalars(s_col, q_col, tagp):
                """mean/rstd per rowchunk from per-partition sums: all-DVE,
                Newton rsqrt (no ACT table, no Sqrt set thrash).
                Returns (m_col [128,RC], rstd [128,RC], nmr [128,RC])."""
                m_col = stp.tile([128, RC], f32, tag=f"m_{tagp}", name=f"m_{tagp}")
                nc.vector.tensor_scalar_mul(m_col[:], s_col[:], INV_E)
                ve = stp.tile([128, RC], f32, tag=f"ve_{tagp}", name=f"ve_{tagp}")
                # ve = q/E - m*m + eps
                msq = stp.tile([128, RC], f32, tag=f"msq_{tagp}", name=f"msq_{tagp}")
                nc.vector.tensor_mul(msq[:], m_col[:], m_col[:])
                nc.vector.tensor_scalar(
                    ve[:], q_col[:], INV_E, EPS, ALU.mult, ALU.add)
                nc.vector.tensor_sub(ve[:], ve[:], msq[:])
                # Newton rsqrt: y0 via int bit-hack, then 2 iterations
                yr = stp.tile([128, RC], f32, tag=f"yr_{tagp}", name=f"yr_{tagp}")
                nc.vector.tensor_scalar(
                    yr.bitcast(dt.int32)[:], ve.bitcast(dt.int32)[:],
                    1, -1, ALU.arith_shift_right, ALU.bitwise_xor)
                nc.vector.tensor_scalar_add(
                    yr.bitcast(dt.int32)[:], yr.bitcast(dt.int32)[:], MAGIC)
                t0 = stp.tile([128, RC], f32, tag=f"t0_{tagp}", name=f"t0_{tagp}")
                for _ in range(2):
                    nc.vector.tensor_mul(t0[:], yr[:], yr[:])
                    nc.vector.tensor_mul(t0[:], t0[:], ve[:])
                    nc.vector.tensor_scalar(
                        t0[:], t0[:], -0.5, 1.5, ALU.mult, ALU.add)
                    nc.vector.tensor_mul(yr[:], yr[:], t0[:])
                nmr = stp.tile([128, RC], f32, tag=f"nmr_{tagp}", name=f"nmr_{tagp}")
                nc.vector.scalar_tensor_tensor(
                    nmr[:], m_col[:], -1.0, yr[:], ALU.mult, ALU.mult)
                return m_col, yr, nmr

            def body(_iv=None):
                for st in range(nst):
                    r0 = st * ST
                    xT_sb, xp_sb = {}, {}
                    for m in mods:
                        xT_sb[m] = xTp.tile([128, EC, ST], bf, tag="xT", name=f"xT_{m}_sb")
                        nc.sync.dma_start(
                            xT_sb[m][:],
                            xT[m][:, r0:r0 + ST].rearrange("(kc p) n -> p kc n", p=128))
                        xp_sb[m] = xpp.tile([128, RC, E], f32, tag="xp", name=f"xp_{m}_sb")
                        nc.sync.dma_start(
                            xp_sb[m][:],
                            xp[m][r0:r0 + ST, :].rearrange("(r p) f -> p r f", p=128))

                    for m in mods:          # branch producing out[m]
                        o = other[m]
                        z1s, y1s = [], []
                        s1_col = stp.tile([128, RC], f32, tag="s1c", name="s1_col")
                        q1_col = stp.tile([128, RC], f32, tag="q1c", name="q1_col")
                        y1T_sb = y1Tp.tile([128, EC, ST], bf, tag="y1T")
                        # --- attn + z1 + stats per rowchunk ---
                        for r in range(RC):
                            p_a = pa_pool.tile([128, E], f32, tag="pa")
                            for kc in range(EC):
                                nc.tensor.matmul(
                                    p_a[:],
                                    xT_sb[o][:, kc, r * 128:(r + 1) * 128],
                                    wefft_sb[m][:, kc, :],
                                    start=(kc == 0), stop=(kc == EC - 1))
                            z1 = zp.tile([128, E], f32, tag="z1")
                            # z1 = attn + x' and row-sum in one DVE op
                            nc.vector.scalar_tensor_tensor(
                                z1[:], p_a[:], 0.0, xp_sb[m][:, r, :],
                                ALU.bypass, ALU.add,
                                accum_out=s1_col[:, r:r + 1])
                            # sum(z1^2) on GPSIMD (scratch out, accum kept)
                            scr = scrp.tile([128, E], f32, tag="scr")
                            nc.gpsimd.scalar_tensor_tensor(
                                scr[:], z1[:], 0.0, z1[:],
                                ALU.bypass, ALU.mult,
                                accum_out=q1_col[:, r:r + 1])
                            z1s.append(z1)
                        m1, r1, _ = ln_scalars(s1_col, q1_col, "l1")
                        for r in range(RC):
                            y1r = yp.tile([128, E], f32, tag="y1")
                            nc.vector.tensor_scalar(
                                y1r[:], z1s[r][:], m1[:, r:r + 1], r1[:, r:r + 1],
                                ALU.subtract, ALU.mult)
                            y1s.append(y1r)
                            y1b = ybp.tile([128, E], bf, tag="y1b")
                            nc.scalar.copy(y1b[:], y1r[:])
                            nc.sync.dma_start_transpose(
                                y1T_sb[:, :, r * 128:(r + 1) * 128], y1b[:])
                        # --- FFN1 + gelu (feature-major out) ---
                        h_sb = hp.tile([128, FC, ST], bf, tag="h")
                        for mc in range(FC):
                            p_u = pu_pool.tile([128, ST], f32, tag="pu")
                            for kc in range(EC):
                                nc.tensor.matmul(
                                    p_u[:],
                                    w1t_sb[m][:, kc, mc * 128:(mc + 1) * 128],
                                    y1T_sb[:, kc, :],
                                    start=(kc == 0), stop=(kc == EC - 1))
                            nc.scalar.activation(
                                h_sb[:, mc, :], p_u[:],
                                mybir.ActivationFunctionType.Gelu if real_gelu
                                else mybir.ActivationFunctionType.Identity,
                                bias=b1_sb[m][:, mc:mc + 1])
                        # --- FFN2 + z2 + stats per rowchunk ---
                        z2s = []
                        s2_col = stp.tile([128, RC], f32, tag="s2c", name="s2_col")
                        q2_col = stp.tile([128, RC], f32, tag="q2c", name="q2_col")
                        for r in range(RC):
                            p_f = pf_pool.tile([128, E], f32, tag="pf")
                            for kc in range(FC):
                                nc.tensor.matmul(
                                    p_f[:],
                                    h_sb[:, kc, r * 128:(r + 1) * 128],
                                    w2t_sb[m][:, kc, :],
                                    start=(kc == 0), stop=False)
                            nc.tensor.matmul(
                                p_f[:], ones_sb[:], b2_sb[m][:],
                                start=False, stop=True)
                            z2 = zp.tile([128, E], f32, tag="z2")
                            nc.vector.scalar_tensor_tensor(
                                z2[:], p_f[:], 0.0, y1s[r][:],
                                ALU.bypass, ALU.add,
                                accum_out=s2_col[:, r:r + 1])
                            scr = scrp.tile([128, E], f32, tag="scr")
                            nc.gpsimd.scalar_tensor_tensor(
                                scr[:], z2[:], 0.0, z2[:],
                                ALU.bypass, ALU.mult,
                                accum_out=q2_col[:, r:r + 1])
                            z2s.append(z2)
                        _, r2, nmr2 = ln_scalars(s2_col, q2_col, "l2")
                        for r in range(RC):
                            y2 = y2p.tile([128, E], f32, tag="y2")
                            # y2 = rstd*z2 - mean*rstd on ACT (Copy in gelu set)
                            nc.scalar.activation(
                                y2[:], z2s[r][:],
                                mybir.ActivationFunctionType.Identity,
                                bias=nmr2[:, r:r + 1], scale=r2[:, r:r + 1])
                            nc.sync.dma_start(
                                out[m][r0 + r * 128:r0 + (r + 1) * 128, :], y2[:])

            if reps == 1:
                body()
            else:
                with tc.For_i(0, reps, 1) as iv:
                    body(iv)

    nc.compile()
    return nc


def _host_prepare(inputs):
    """Fold weights, shard the batch, build per-core input maps."""
    f64 = np.float64

    def fold(p):
        Wv, bv = inputs[f"{p}_Wv"], inputs[f"{p}_bv"]
        Wo, bo = inputs[f"{p}_Wo"], inputs[f"{p}_bo"]
        weff = (Wo.astype(f64) @ Wv.astype(f64)).astype(np.float32)
        beff = (Wo.astype(f64) @ bv.astype(f64) + bo.astype(f64)).astype(np.float32)
        return weff, beff

    weff_i2t, beff_i2t = fold("i2t")   # img branch: img queries text context
    weff_t2i, beff_t2i = fold("t2i")   # text branch

    const = {}
    br_w = {"img": (weff_i2t, beff_i2t, "ffn_img"),
            "text": (weff_t2i, beff_t2i, "ffn_text")}
    for m, (weff, beff, ffn) in br_w.items():
        const[f"wefft_{m}"] = np.ascontiguousarray(weff.T).astype(BF16)
        const[f"w1t_{m}"] = np.ascontiguousarray(inputs[f"{ffn}_W1"].T).astype(BF16)
        const[f"w2t_{m}"] = np.ascontiguousarray(inputs[f"{ffn}_W2"].T).astype(BF16)
        const[f"b1_{m}"] = np.ascontiguousarray(
            inputs[f"{ffn}_b1"].reshape(FC, 128).T).astype(np.float32)
        const[f"b2_{m}"] = inputs[f"{ffn}_b2"].reshape(1, E).astype(BF16)

    text = np.asarray(inputs["text_feat"], dtype=np.float32)
    img = np.asarray(inputs["img_feat"], dtype=np.float32)
    in_maps = []
    for c in range(N_CORES):
        s = slice(c * RPC, (c + 1) * RPC)
        ts, is_ = text[s], img[s]
        m = dict(const)
        m["xT_text"] = np.ascontiguousarray(ts.T).astype(BF16)
        m["xT_img"] = np.ascontiguousarray(is_.T).astype(BF16)
        m["xp_img"] = is_ + beff_i2t[None, :]
        m["xp_text"] = ts + beff_t2i[None, :]
        in_maps.append(m)
    return in_maps


def _ln_affine_trivial(inputs):
    for n in ("ln1i", "ln1t", "ln2i", "ln2t"):
        if not np.allclose(inputs[f"{n}_g"], 1.0, atol=0.0):
            return False
        if not np.allclose(inputs[f"{n}_b"], 0.0, atol=0.0):
            return False
    return True


def _reference_numpy(inputs):
    """Exact numpy fallback (only used if LN affine params are nontrivial)."""
    from scipy.special import erf  # pragma: no cover

    def ln(x, g, b, eps=EPS):
        m = x.mean(-1, keepdims=True)
        xc = x - m
        v = (xc * xc).mean(-1, keepdims=True)
        return xc / np.sqrt(v + eps) * g + b

    def mha1(q, kv, p):
        vh = kv @ inputs[f"{p}_Wv"].T + inputs[f"{p}_bv"]
        return vh @ inputs[f"{p}_Wo"].T + inputs[f"{p}_bo"]

    def ffn(x, p):
        u = x @ inputs[f"{p}_W1"].T + inputs[f"{p}_b1"]
        h = 0.5 * u * (1.0 + erf(u / np.sqrt(2.0)))
        return h @ inputs[f"{p}_W2"].T + inputs[f"{p}_b2"]

    text, img = inputs["text_feat"], inputs["img_feat"]
    img_out = ln(img + mha1(img, text, "i2t"), inputs["ln1i_g"], inputs["ln1i_b"])
    text_out = ln(text + mha1(text, img, "t2i"), inputs["ln1t_g"], inputs["ln1t_b"])
    img_out = ln(img_out + ffn(img_out, "ffn_img"), inputs["ln2i_g"], inputs["ln2i_b"])
    text_out = ln(text_out + ffn(text_out, "ffn_text"),
                  inputs["ln2t_g"], inputs["ln2t_b"])
    return text_out.astype(np.float32), img_out.astype(np.float32)


def kernel(**inputs):
    inputs = {k: np.asarray(v) for k, v in inputs.items()}
    if not _ln_affine_trivial(inputs):
        return _reference_numpy(inputs)

    from concourse.bass_utils import run_bass_kernel_spmd

    key = ("main", RPC, 1)
    if key not in _BUILD_CACHE:
        _BUILD_CACHE[key] = _build_program(RPC, reps=1)
    nc = _BUILD_CACHE[key]

    in_maps = _host_prepare(inputs)
    res = run_bass_kernel_spmd(nc, in_maps, core_ids=list(range(N_CORES)))
    text_out = np.concatenate([res.results[c]["out_text"] for c in range(N_CORES)], 0)
    img_out = np.concatenate([res.results[c]["out_img"] for c in range(N_CORES)], 0)
    return text_out.astype(np.float32), img_out.astype(np.float32)


# revision 13
# speedup vs baseline: 3.2672x; 1.3719x over previous
"""CrossModalAttention Trainium2 Bass kernel (8-core data parallel).

Math notes (matches the fp32 jax reference exactly up to float rounding):
  - nn.MultiheadAttention with q_len = kv_len = 1: softmax over a single key
    is identically 1.0, so attn(q, kv) = (kv @ Wv.T + bv) @ Wo.T + bo
    = kv @ (Wo @ Wv).T + (Wo @ bv + bo).  Wq / Wk drop out entirely.
  - Per branch (img / text):
        z1 = x + x_other @ Weff.T + beff
        y1 = LN1(z1)                       (gamma=1, beta=0 in this problem)
        z2 = y1 + gelu(y1 @ W1.T + b1) @ W2.T + b2
        y2 = LN2(z2)                       -> output
Device layout: activations row-major (rows on SBUF partitions), LN via
bn_stats/bn_aggr + fused tensor_scalar.  Matmuls are activation-stationary
(lhsT = transposed activations): attn uses host-pre-transposed x, FFN2 uses
the feature-major gelu output directly, only y1 is transposed on device
(bf16 DMA transpose).  Matmul inputs bf16, everything else fp32.
"""

import os
import sys

import numpy as np

for _p in ("/opt/trn_rl_repo", "/root/.axon_site/_ro/trn_rl_repo",
           "/root/.axon_site", "/root/.axon_site/_ro/pypackages"):
    if os.path.isdir(_p) and _p not in sys.path:
        sys.path.append(_p)

import ml_dtypes

BF16 = ml_dtypes.bfloat16

B, E, FF, H = 65536, 512, 1024, 8
N_CORES = 8
RPC = B // N_CORES          # rows per core
ST = 512                    # supertile rows
RC = ST // 128              # rowchunks per supertile
EC = E // 128               # feature chunks (4)
FC = FF // 128              # ffn feature chunks (8)
EPS = 1e-5

_BUILD_CACHE = {}


def _build_program(rows, reps=1, real_gelu=True):
    """Build + compile the Bass program for one core processing `rows` rows."""
    import concourse.bass as bass  # noqa: F401
    import concourse.mybir as mybir
    from concourse import bacc, tile

    dt = mybir.dt
    f32, bf = dt.float32, dt.bfloat16
    nst = rows // ST
    assert rows % ST == 0

    nc = bacc.Bacc("TRN2", target_bir_lowering=False, debug=False,
                   enable_asserts=False, num_devices=1)

    def din(name, shape, d=f32):
        return nc.dram_tensor(name, shape, d, kind="ExternalInput").ap()

    mods = ("text", "img")
    xT = {m: din(f"xT_{m}", (E, rows), bf) for m in mods}
    xp = {m: din(f"xp_{m}", (rows, E), f32) for m in mods}
    wefft = {m: din(f"wefft_{m}", (E, E), bf) for m in mods}
    w1t = {m: din(f"w1t_{m}", (E, FF), bf) for m in mods}
    w2t = {m: din(f"w2t_{m}", (FF, E), bf) for m in mods}
    b1 = {m: din(f"b1_{m}", (128, FC), f32) for m in mods}
    b2 = {m: din(f"b2_{m}", (1, E), bf) for m in mods}
    out = {m: nc.dram_tensor(f"out_{m}", (rows, E), f32, kind="ExternalOutput").ap()
           for m in mods}
    other = {"img": "text", "text": "img"}

    with tile.TileContext(nc) as tc:
        with (
            tc.tile_pool(name="wpool", bufs=1) as wpool,
            tc.tile_pool(name="xTp", bufs=4) as xTp,
            tc.tile_pool(name="xpp", bufs=4) as xpp,
            tc.tile_pool(name="y1Tp", bufs=3) as y1Tp,
            tc.tile_pool(name="hp", bufs=2) as hp,
            tc.tile_pool(name="zp", bufs=6) as zp,
            tc.tile_pool(name="scrp", bufs=2) as scrp,
            tc.tile_pool(name="yp", bufs=12) as yp,
            tc.tile_pool(name="ybp", bufs=4) as ybp,
            tc.tile_pool(name="y2p", bufs=4) as y2p,
            tc.tile_pool(name="stp", bufs=16) as stp,
            tc.tile_pool(name="pa", bufs=3, space="PSUM") as pa_pool,
            tc.tile_pool(name="pu", bufs=3, space="PSUM") as pu_pool,
            tc.tile_pool(name="pf", bufs=2, space="PSUM") as pf_pool,
        ):
            # ---- resident weights ----
            wefft_sb, w1t_sb, w2t_sb, b1_sb, b2_sb = {}, {}, {}, {}, {}
            for m in mods:
                wefft_sb[m] = wpool.tile([128, EC, E], bf, tag=f"wefft_{m}", name=f"wefft_{m}_sb")
                nc.sync.dma_start(
                    wefft_sb[m][:], wefft[m].rearrange("(kc p) n -> p kc n", p=128))
                w1t_sb[m] = wpool.tile([128, EC, FF], bf, tag=f"w1t_{m}", name=f"w1t_{m}_sb")
                nc.sync.dma_start(
                    w1t_sb[m][:], w1t[m].rearrange("(kc p) n -> p kc n", p=128))
                w2t_sb[m] = wpool.tile([128, FC, E], bf, tag=f"w2t_{m}", name=f"w2t_{m}_sb")
                nc.sync.dma_start(
                    w2t_sb[m][:], w2t[m].rearrange("(kc p) n -> p kc n", p=128))
                b1_sb[m] = wpool.tile([128, FC], f32, tag=f"b1_{m}", name=f"b1_{m}_sb")
                nc.sync.dma_start(b1_sb[m][:], b1[m])
                b2_sb[m] = wpool.tile([1, E], bf, tag=f"b2_{m}", name=f"b2_{m}_sb")
                nc.sync.dma_start(b2_sb[m][:], b2[m])
            ones_sb = wpool.tile([1, 128], bf, tag="ones")
            nc.vector.memset(ones_sb[:], 1.0)

            ALU = mybir.AluOpType
            INV_E = 1.0 / E
            MAGIC = 0x5F3759DF + 1   # two's-complement: magic - x = ~x + (magic+1)

            def ln_scalars(s_col, q_col, tagp):
                """mean/rstd per rowchunk from per-partition sums: all-DVE,
                Newton rsqrt (no ACT table, no Sqrt set thrash).
                Returns (m_col [128,RC], rstd [128,RC], nmr [128,RC])."""
                m_col = stp.tile([128, RC], f32, tag=f"m_{tagp}", name=f"m_{tagp}")
                nc.vector.tensor_scalar_mul(m_col[:], s_col[:], INV_E)
                ve = stp.tile([128, RC], f32, tag=f"ve_{tagp}", name=f"ve_{tagp}")
                # ve = q/E - m*m + eps
                msq = stp.tile([128, RC], f32, tag=f"msq_{tagp}", name=f"msq_{tagp}")
                nc.vector.tensor_mul(msq[:], m_col[:], m_col[:])
                nc.vector.tensor_scalar(
                    ve[:], q_col[:], INV_E, EPS, ALU.mult, ALU.add)
                nc.vector.tensor_sub(ve[:], ve[:], msq[:])
                # Newton rsqrt: y0 via int bit-hack, then 2 iterations
                yr = stp.tile([128, RC], f32, tag=f"yr_{tagp}", name=f"yr_{tagp}")
                nc.vector.tensor_scalar(
                    yr.bitcast(dt.int32)[:], ve.bitcast(dt.int32)[:],
                    1, -1, ALU.arith_shift_right, ALU.bitwise_xor)
                nc.vector.tensor_scalar_add(
                    yr.bitcast(dt.int32)[:], yr.bitcast(dt.int32)[:], MAGIC)
                t0 = stp.tile([128, RC], f32, tag=f"t0_{tagp}", name=f"t0_{tagp}")
                for _ in range(2):
                    nc.vector.tensor_mul(t0[:], yr[:], yr[:])
                    nc.vector.tensor_mul(t0[:], t0[:], ve[:])
                    nc.vector.tensor_scalar(
                        t0[:], t0[:], -0.5, 1.5, ALU.mult, ALU.add)
                    nc.vector.tensor_mul(yr[:], yr[:], t0[:])
                return m_col, yr, None

            def phase_load(st):
                r0 = st * ST
                xT_sb, xp_sb = {}, {}
                for m in mods:
                    xT_sb[m] = xTp.tile([128, EC, ST], bf, tag="xT", name=f"xT_{m}_sb")
                    nc.sync.dma_start(
                        xT_sb[m][:],
                        xT[m][:, r0:r0 + ST].rearrange("(kc p) n -> p kc n", p=128))
                    xp_sb[m] = xpp.tile([128, RC, E], f32, tag="xp", name=f"xp_{m}_sb")
                    nc.sync.dma_start(
                        xp_sb[m][:],
                        xp[m][r0:r0 + ST, :].rearrange("(r p) f -> p r f", p=128))
                return xT_sb, xp_sb

            def phase_A(st, xT_sb, xp_sb):
                sbr = {}
                for m in mods:          # attn + LN1 (both branches)
                    o = other[m]
                    z1s, y1s = [], []
                    s1_col = stp.tile([128, RC], f32, tag="s1c", name=f"s1_col_{m}")
                    q1_col = stp.tile([128, RC], f32, tag="q1c", name=f"q1_col_{m}")
                    y1T_sb = y1Tp.tile([128, EC, ST], bf, tag="y1T", name=f"y1T_{m}")
                    for r in range(RC):
                        p_a = pa_pool.tile([128, E], f32, tag="pa")
                        for kc in range(EC):
                            nc.tensor.matmul(
                                p_a[:],
                                xT_sb[o][:, kc, r * 128:(r + 1) * 128],
                                wefft_sb[m][:, kc, :],
                                start=(kc == 0), stop=(kc == EC - 1))
                        z1 = zp.tile([128, E], f32, tag="z1")
                        # z1 = attn + x' and row-sum in one DVE op
                        nc.vector.scalar_tensor_tensor(
                            z1[:], p_a[:], 0.0, xp_sb[m][:, r, :],
                            ALU.bypass, ALU.add,
                            accum_out=s1_col[:, r:r + 1])
                        # sum(z1^2) on ACT (Square is in the gelu set)
                        scr = scrp.tile([128, E], f32, tag="scr")
                        nc.scalar.activation(
                            scr[:], z1[:],
                            mybir.ActivationFunctionType.Square,
                            accum_out=q1_col[:, r:r + 1])
                        z1s.append(z1)
                    m1, r1, _ = ln_scalars(s1_col, q1_col, "l1")
                    for r in range(RC):
                        y1r = yp.tile([128, E], f32, tag="y1")
                        nc.vector.tensor_scalar(
                            y1r[:], z1s[r][:], m1[:, r:r + 1], r1[:, r:r + 1],
                            ALU.subtract, ALU.mult)
                        y1s.append(y1r)
                        y1b = ybp.tile([128, E], bf, tag="y1b")
                        nc.gpsimd.tensor_copy(y1b[:], y1r[:])
                        nc.sync.dma_start_transpose(
                            y1T_sb[:, :, r * 128:(r + 1) * 128], y1b[:])
                    sbr[m] = (y1T_sb, y1s)
                return sbr

            def phase_B(sbr):
                hs = {}
                for m in mods:          # FFN1 + gelu
                    y1T_sb, _ = sbr[m]
                    h_sb = hp.tile([128, FC, ST], bf, tag="h", name=f"h_{m}")
                    for mc in range(FC):
                        p_u = pu_pool.tile([128, ST], f32, tag="pu")
                        for kc in range(EC):
                            nc.tensor.matmul(
                                p_u[:],
                                w1t_sb[m][:, kc, mc * 128:(mc + 1) * 128],
                                y1T_sb[:, kc, :],
                                start=(kc == 0), stop=(kc == EC - 1))
                        nc.scalar.activation(
                            h_sb[:, mc, :], p_u[:],
                            mybir.ActivationFunctionType.Gelu if real_gelu
                            else mybir.ActivationFunctionType.Identity,
                            bias=b1_sb[m][:, mc:mc + 1])
                    hs[m] = h_sb
                return hs

            def phase_C(st, sbr, hs):
                r0 = st * ST
                for m in mods:          # FFN2 + LN2 + store
                    h_sb = hs[m]
                    _, y1s = sbr[m]
                    z2s = []
                    s2_col = stp.tile([128, RC], f32, tag="s2c", name=f"s2_col_{m}")
                    q2_col = stp.tile([128, RC], f32, tag="q2c", name=f"q2_col_{m}")
                    for r in range(RC):
                        p_f = pf_pool.tile([128, E], f32, tag="pf")
                        for kc in range(FC):
                            nc.tensor.matmul(
                                p_f[:],
                                h_sb[:, kc, r * 128:(r + 1) * 128],
                                w2t_sb[m][:, kc, :],
                                start=(kc == 0), stop=False)
                        nc.tensor.matmul(
                            p_f[:], ones_sb[:], b2_sb[m][:],
                            start=False, stop=True)
                        z2 = zp.tile([128, E], f32, tag="z2")
                        nc.vector.scalar_tensor_tensor(
                            z2[:], p_f[:], 0.0, y1s[r][:],
                            ALU.bypass, ALU.add,
                            accum_out=s2_col[:, r:r + 1])
                        scr = scrp.tile([128, E], f32, tag="scr")
                        nc.scalar.activation(
                            scr[:], z2[:],
                            mybir.ActivationFunctionType.Square,
                            accum_out=q2_col[:, r:r + 1])
                        z2s.append(z2)
                    m2, r2, _ = ln_scalars(s2_col, q2_col, "l2")
                    for r in range(RC):
                        y2 = y2p.tile([128, E], f32, tag="y2")
                        nc.vector.tensor_scalar(
                            y2[:], z2s[r][:], m2[:, r:r + 1], r2[:, r:r + 1],
                            ALU.subtract, ALU.mult)
                        nc.sync.dma_start(
                            out[m][r0 + r * 128:r0 + (r + 1) * 128, :], y2[:])

            def body(_iv=None):
                pending = None
                for st in range(nst):
                    xT_sb, xp_sb = phase_load(st)
                    sbr = phase_A(st, xT_sb, xp_sb)
                    if pending is not None:
                        pst, psbr = pending
                        phase_C(pst, psbr, phase_B(psbr))
                    pending = (st, sbr)
                pst, psbr = pending
                phase_C(pst, psbr, phase_B(psbr))

            if reps == 1:
                body()
            else:
                with tc.For_i(0, reps, 1) as iv:
                    body(iv)

    nc.compile()
    return nc


def _host_prepare(inputs):
    """Fold weights, shard the batch, build per-core input maps."""
    f64 = np.float64

    def fold(p):
        Wv, bv = inputs[f"{p}_Wv"], inputs[f"{p}_bv"]
        Wo, bo = inputs[f"{p}_Wo"], inputs[f"{p}_bo"]
        weff = (Wo.astype(f64) @ Wv.astype(f64)).astype(np.float32)
        beff = (Wo.astype(f64) @ bv.astype(f64) + bo.astype(f64)).astype(np.float32)
        return weff, beff

    weff_i2t, beff_i2t = fold("i2t")   # img branch: img queries text context
    weff_t2i, beff_t2i = fold("t2i")   # text branch

    const = {}
    br_w = {"img": (weff_i2t, beff_i2t, "ffn_img"),
            "text": (weff_t2i, beff_t2i, "ffn_text")}
    for m, (weff, beff, ffn) in br_w.items():
        const[f"wefft_{m}"] = np.ascontiguousarray(weff.T).astype(BF16)
        const[f"w1t_{m}"] = np.ascontiguousarray(inputs[f"{ffn}_W1"].T).astype(BF16)
        const[f"w2t_{m}"] = np.ascontiguousarray(inputs[f"{ffn}_W2"].T).astype(BF16)
        const[f"b1_{m}"] = np.ascontiguousarray(
            inputs[f"{ffn}_b1"].reshape(FC, 128).T).astype(np.float32)
        const[f"b2_{m}"] = inputs[f"{ffn}_b2"].reshape(1, E).astype(BF16)

    text = np.asarray(inputs["text_feat"], dtype=np.float32)
    img = np.asarray(inputs["img_feat"], dtype=np.float32)
    in_maps = []
    for c in range(N_CORES):
        s = slice(c * RPC, (c + 1) * RPC)
        ts, is_ = text[s], img[s]
        m = dict(const)
        m["xT_text"] = np.ascontiguousarray(ts.T).astype(BF16)
        m["xT_img"] = np.ascontiguousarray(is_.T).astype(BF16)
        m["xp_img"] = is_ + beff_i2t[None, :]
        m["xp_text"] = ts + beff_t2i[None, :]
        in_maps.append(m)
    return in_maps


def _ln_affine_trivial(inputs):
    for n in ("ln1i", "ln1t", "ln2i", "ln2t"):
        if not np.allclose(inputs[f"{n}_g"], 1.0, atol=0.0):
            return False
        if not np.allclose(inputs[f"{n}_b"], 0.0, atol=0.0):
            return False
    return True


def _reference_numpy(inputs):
    """Exact numpy fallback (only used if LN affine params are nontrivial)."""
    from scipy.special import erf  # pragma: no cover

    def ln(x, g, b, eps=EPS):
        m = x.mean(-1, keepdims=True)
        xc = x - m
        v = (xc * xc).mean(-1, keepdims=True)
        return xc / np.sqrt(v + eps) * g + b

    def mha1(q, kv, p):
        vh = kv @ inputs[f"{p}_Wv"].T + inputs[f"{p}_bv"]
        return vh @ inputs[f"{p}_Wo"].T + inputs[f"{p}_bo"]

    def ffn(x, p):
        u = x @ inputs[f"{p}_W1"].T + inputs[f"{p}_b1"]
        h = 0.5 * u * (1.0 + erf(u / np.sqrt(2.0)))
        return h @ inputs[f"{p}_W2"].T + inputs[f"{p}_b2"]

    text, img = inputs["text_feat"], inputs["img_feat"]
    img_out = ln(img + mha1(img, text, "i2t"), inputs["ln1i_g"], inputs["ln1i_b"])
    text_out = ln(text + mha1(text, img, "t2i"), inputs["ln1t_g"], inputs["ln1t_b"])
    img_out = ln(img_out + ffn(img_out, "ffn_img"), inputs["ln2i_g"], inputs["ln2i_b"])
    text_out = ln(text_out + ffn(text_out, "ffn_text"),
                  inputs["ln2t_g"], inputs["ln2t_b"])
    return text_out.astype(np.float32), img_out.astype(np.float32)


def _device_run(inputs):
    from concourse.bass_utils import run_bass_kernel_spmd

    key = ("main", RPC, 1)
    if key not in _BUILD_CACHE:
        _BUILD_CACHE[key] = _build_program(RPC, reps=1)
    nc = _BUILD_CACHE[key]

    in_maps = _host_prepare(inputs)
    res = run_bass_kernel_spmd(nc, in_maps, core_ids=list(range(N_CORES)))
    text_out = np.concatenate([res.results[c]["out_text"] for c in range(N_CORES)], 0)
    img_out = np.concatenate([res.results[c]["out_img"] for c in range(N_CORES)], 0)
    return text_out.astype(np.float32), img_out.astype(np.float32)


def _axon_available():
    try:
        import jax

        return any(d.platform == "axon" for d in jax.devices())
    except Exception:
        return False


def _subproc_entry(in_path, out_path):
    z = np.load(in_path)
    inputs = {k: z[k] for k in z.files}
    text_out, img_out = _device_run(inputs)
    np.savez(out_path, text_out=text_out, img_out=img_out)


def _device_run_subprocess(inputs):
    """Run the device path in a child process with a clean jax platform env.

    Needed when the caller's process pinned JAX_PLATFORMS=cpu (e.g. to run
    the jax reference on CPU), which would hide the axon/neuron devices."""
    import subprocess
    import tempfile

    d = tempfile.mkdtemp(prefix="cmattn_")
    in_path, out_path = os.path.join(d, "in.npz"), os.path.join(d, "out.npz")
    np.savez(in_path, **inputs)
    here = os.path.dirname(os.path.abspath(__file__))
    env = dict(os.environ)
    env.pop("JAX_PLATFORMS", None)
    code = (
        "import sys; sys.path.insert(0, %r); import kernel; "
        "kernel._subproc_entry(%r, %r)" % (here, in_path, out_path)
    )
    subprocess.run([sys.executable, "-c", code], check=True, env=env)
    z = np.load(out_path)
    return z["text_out"], z["img_out"]


def kernel(**inputs):
    inputs = {k: np.asarray(v) for k, v in inputs.items()}
    if not _ln_affine_trivial(inputs):
        return _reference_numpy(inputs)
    if _axon_available():
        return _device_run(inputs)
    return _device_run_subprocess(inputs)
